# revision 8
# baseline (speedup 1.0000x reference)
"""Self-contained Trainium2 (8-core) kernel for nn_DecoderBlock_82660940579418.

Decoder block: self-attn -> LN -> cross-attn -> LN -> FFN -> LN, with the
reference's softmax over the *query* axis.

Sharding: attention heads (2 per core) for QKV/scores/AV; tokens (256 per
core) for out-projections, LayerNorms and the FFN. Cross-core traffic is
four 1 MB AllToAlls (bf16). Compute dtype bf16 on TensorE, fp32 PSUM,
fp32 residual stream and LayerNorm.
"""

import os
import sys

for _p in ("/opt/trn_rl_repo", "/root/.axon_site/_ro/trn_rl_repo"):
    if os.path.isdir(_p) and _p not in sys.path:
        sys.path.append(_p)

import numpy as np
import ml_dtypes

B, S, D, H, DH, DFF = 2, 1024, 2048, 16, 128, 8192
NCORES = 8
T = B * S            # 2048 tokens, index t = b*S + s
TSH = T // NCORES    # 256 tokens per core
HPC = H // NCORES    # 2 heads per core
DT = D // 128        # 16 feature tiles
FT = DFF // 128      # 64 ffn-hidden tiles
BF16 = ml_dtypes.bfloat16

_CACHE = {}


def _build(debug_taps=False):
    import concourse.bass as bass
    import concourse.mybir as mybir
    import concourse.tile as tile
    from concourse import bacc
    from concourse.masks import make_identity

    f32 = mybir.dt.float32
    bf = mybir.dt.bfloat16
    Act = mybir.ActivationFunctionType
    Alu = mybir.AluOpType

    nc = bacc.Bacc("TRN2", target_bir_lowering=False, debug=False,
                   num_devices=NCORES)

    def din(name, shape, dt):
        return nc.dram_tensor(name, shape, dt, kind="ExternalInput")

    # ---- external inputs (per core) ----
    xT = din("xT", [D, T], bf)           # x^T, feature-major, full
    encT = din("encT", [D, T], bf)       # encoder_output^T, full
    xsh = din("xsh", [TSH, D], f32)      # my token shard of x, + bp1 folded
    wq1 = din("wq1", [D, HPC * DH], bf)  # my heads, pre-scaled by DH^-0.5
    wk1 = din("wk1", [D, HPC * DH], bf)
    wv1 = din("wv1", [D, HPC * DH], bf)
    bq1 = din("bq1", [DH, HPC], f32)     # pre-scaled
    bk1 = din("bk1", [DH, HPC], f32)
    bv1r = din("bv1r", [1, HPC * DH], f32)
    wp1 = din("wp1", [D, D], bf)
    wq2 = din("wq2", [D, HPC * DH], bf)  # pre-scaled
    bq2 = din("bq2", [DH, HPC], f32)
    wk2a = din("wk2a", [D, D], bf)       # all heads
    bk2a = din("bk2a", [DH, H], f32)
    wv2a = din("wv2a", [D, D], bf)
    bv2r = din("bv2r", [1, D], f32)
    wp2 = din("wp2", [D, D], bf)
    bp2r = din("bp2r", [1, D], f32)
    wf1 = din("wf1", [D, DFF], bf)
    bf1c = din("bf1c", [DH, FT], f32)
    wf2 = din("wf2", [DFF, D], bf)
    bf2r = din("bf2r", [1, D], f32)
    g1r = din("g1r", [1, D], f32)
    b1r = din("b1r", [1, D], f32)
    g2r = din("g2r", [1, D], f32)
    b2r = din("b2r", [1, D], f32)
    g3r = din("g3r", [1, D], f32)
    b3r = din("b3r", [1, D], f32)

    osh = nc.dram_tensor("osh", [TSH, D], f32, kind="ExternalOutput")

    taps = {}
    if debug_taps:
        taps["qT1"] = nc.dram_tensor("tap_qT1", [DH, HPC, T], f32, kind="ExternalOutput")
        taps["den1"] = nc.dram_tensor("tap_den1", [DH, 4, 8], f32, kind="ExternalOutput")
        taps["outT1"] = nc.dram_tensor("tap_outT1", [DH, HPC, T], f32, kind="ExternalOutput")
        taps["x1"] = nc.dram_tensor("tap_x1", [TSH, D], f32, kind="ExternalOutput")
        taps["x2"] = nc.dram_tensor("tap_x2", [TSH, D], f32, kind="ExternalOutput")
        taps["kT2"] = nc.dram_tensor("tap_kT2", [DH, HPC, T], f32, kind="ExternalOutput")
        taps["v2"] = nc.dram_tensor("tap_v2", [DH, 16, HPC * DH], f32, kind="ExternalOutput")

    # rearranged DRAM views: [128, tile, free]
    xT3 = xT.ap().rearrange("(o p) t -> p o t", p=128)
    encT3 = encT.ap().rearrange("(o p) t -> p o t", p=128)
    wq1r = wq1.ap().rearrange("(o p) e -> p o e", p=128)
    wk1r = wk1.ap().rearrange("(o p) e -> p o e", p=128)
    wv1r = wv1.ap().rearrange("(o p) e -> p o e", p=128)
    wq2r = wq2.ap().rearrange("(o p) e -> p o e", p=128)
    wk2r = wk2a.ap().rearrange("(o p) f -> p o f", p=128)
    wv2r = wv2a.ap().rearrange("(o p) f -> p o f", p=128)
    wp1r = wp1.ap().rearrange("(o p) d -> p o d", p=128)
    wp2r = wp2.ap().rearrange("(o p) d -> p o d", p=128)
    wf1r = wf1.ap().rearrange("(o p) f -> p o f", p=128)
    wf2r = wf2.ap().rearrange("(o p) d -> p o d", p=128)
    xshr = xsh.ap().rearrange("(o p) d -> p o d", p=128)
    oshr = osh.ap().rearrange("(o p) d -> p o d", p=128)

    def bcast_row(row_t, lo, n, p=128):
        ap = row_t.ap()
        st = ap.ap[-1][0]
        return bass.AP(tensor=ap.tensor, offset=ap.offset + lo * st,
                       ap=[[0, p], [st, n]])

    with tile.TileContext(nc) as tc:
        import contextlib
        ctx = contextlib.ExitStack()
        with ctx:
            pool = ctx.enter_context(tc.tile_pool(name="persist", bufs=1))
            qtp = ctx.enter_context(tc.tile_pool(name="qtp", bufs=1))
            ktp = ctx.enter_context(tc.tile_pool(name="ktp", bufs=1))
            otp = ctx.enter_context(tc.tile_pool(name="otp", bufs=1))
            v1p = ctx.enter_context(tc.tile_pool(name="v1p", bufs=1))
            xtp = ctx.enter_context(tc.tile_pool(name="xtp", bufs=1))
            yp = ctx.enter_context(tc.tile_pool(name="yp", bufs=2))
            expp = ctx.enter_context(tc.tile_pool(name="expp", bufs=3))
            htp = ctx.enter_context(tc.tile_pool(name="htp", bufs=2))
            xblk = ctx.enter_context(tc.tile_pool(name="xblk", bufs=2))
            xblk1 = ctx.enter_context(tc.tile_pool(name="xblk1", bufs=3))
            wrow = ctx.enter_context(tc.tile_pool(name="wrow", bufs=2))
            ott = ctx.enter_context(tc.tile_pool(name="ott", bufs=4))
            vsc = ctx.enter_context(tc.tile_pool(name="vsc", bufs=10))
            kv2 = ctx.enter_context(tc.tile_pool(name="kv2", bufs=3))
            bc = ctx.enter_context(tc.tile_pool(name="bc", bufs=3))
            sm = ctx.enter_context(tc.tile_pool(name="sm", bufs=8))
            resid = ctx.enter_context(tc.tile_pool(name="resid", bufs=2))
            ffap = ctx.enter_context(tc.tile_pool(name="ffap", bufs=1))
            ps = ctx.enter_context(tc.tile_pool(name="ps", bufs=8, space="PSUM"))
            dram = ctx.enter_context(tc.tile_pool(name="dram", bufs=1, space="DRAM"))

            # ---- constants / persistent weights in SBUF ----
            ident = pool.tile([128, 128], f32)
            make_identity(nc, ident[:])
            eps_sb = pool.tile([128, 1], f32)
            nc.vector.memset(eps_sb[:], 1e-5)

            wv1_sb = pool.tile([128, DT, HPC * DH], bf)
            nc.sync.dma_start(wv1_sb[:], wv1r)
            bq1_sb = pool.tile([128, HPC], f32)
            nc.sync.dma_start(bq1_sb[:], bq1.ap()[:])
            bk1_sb = pool.tile([128, HPC], f32)
            nc.sync.dma_start(bk1_sb[:], bk1.ap()[:])
            bq2_sb = pool.tile([128, HPC], f32)
            nc.sync.dma_start(bq2_sb[:], bq2.ap()[:])
            bk2_sb = pool.tile([128, H], f32)
            nc.sync.dma_start(bk2_sb[:], bk2a.ap()[:])
            bf1_sb = pool.tile([128, FT], f32)
            nc.sync.dma_start(bf1_sb[:], bf1c.ap()[:])
            bv1_sb = pool.tile([128, HPC * DH], f32)
            nc.gpsimd.dma_start(bv1_sb[:], bcast_row(bv1r, 0, HPC * DH))

            # ---- DRAM bounce buffers for the 4 AllToAlls ----
            o1_in = dram.tile([NCORES, HPC * DH, TSH], bf)
            o1_out = dram.tile([NCORES, HPC * DH, TSH], bf)
            k2_in = dram.tile([NCORES, HPC * DH, TSH], bf)
            k2_out = dram.tile([NCORES, HPC * DH, TSH], bf)
            v2_in = dram.tile([NCORES, TSH, HPC * DH], bf)
            v2_out = dram.tile([NCORES, TSH, HPC * DH], bf)
            o2_in = dram.tile([NCORES, HPC * DH, TSH], bf)
            o2_out = dram.tile([NCORES, HPC * DH, TSH], bf)
            rg = [list(range(NCORES))]

            def a2a(src, dst):
                nc.gpsimd.collective_compute(
                    "AllToAll", Alu.bypass, replica_groups=rg,
                    ins=[src.opt()], outs=[dst.opt()])

            # ================= helpers =================
            def qk_proj(src3, wdram, b_sb, dst, nproj=2, w2dram=None, b2_sb=None,
                        dst2=None):
                """Feature-major projections for my heads over all tokens.
                dst[:, h, t] = sum_d w[d, h*DH+e] * src[d, t] + b[e, h]."""
                for tc4 in range(T // 512):
                    xtb = xblk.tile([128, DT, 512], bf, tag="xblk")
                    nc.sync.dma_start(xtb[:], src3[:, :, tc4 * 512:(tc4 + 1) * 512])
                    wt1 = xblk1.tile([128, DT, HPC * DH], bf, tag="xblk1")
                    nc.sync.dma_start(wt1[:], wdram)
                    plist = [(wt1, b_sb, dst)]
                    if nproj == 2:
                        wt2 = xblk1.tile([128, DT, HPC * DH], bf, tag="xblk1")
                        nc.sync.dma_start(wt2[:], w2dram)
                        plist.append((wt2, b2_sb, dst2))
                    for h in range(HPC):
                        for (wsb, bsb, dd) in plist:
                            pst = ps.tile([128, 512], f32, tag="ps")
                            for dt in range(DT):
                                nc.tensor.matmul(
                                    pst[:], wsb[:, dt, h * DH:(h + 1) * DH],
                                    xtb[:, dt, :],
                                    start=(dt == 0), stop=(dt == DT - 1))
                            nc.scalar.activation(
                                out=dd[:, h, tc4 * 512:(tc4 + 1) * 512],
                                in_=pst[:], func=Act.Identity,
                                bias=bsb[:, h:h + 1])

            def attn_units(qT, kTs, getv, outT, expect_tap=None):
                """For each (b, h): scores^T=[k,q], exp+denom, scale V rows,
                AV -> outT[:, h, b*S+q] (feature-major)."""
                KT = S // 128   # 8 k tiles per batch
                QC = S // 512   # 2 q chunks per batch
                for b in range(B):
                    for h in range(HPC):
                        expt = [expp.tile([128, KT, 512], bf, tag="expt",
                                          name=f"exp_{b}_{h}_{qc}")
                                for qc in range(QC)]
                        denp = sm.tile([128, KT, QC], f32, tag="denp")
                        for qc in range(QC):
                            for kc in range(KT):
                                pst = ps.tile([128, 512], f32, tag="ps")
                                nc.tensor.matmul(
                                    pst[:],
                                    kTs(h, b * KT + kc),
                                    qT[:, h, b * S + qc * 512: b * S + (qc + 1) * 512],
                                    start=True, stop=True)
                                nc.scalar.activation(
                                    out=expt[qc][:, kc, :],
                                    in_=pst[:], func=Act.Exp,
                                    accum_out=denp[:, kc, qc:qc + 1])
                        den = sm.tile([128, KT], f32, tag="den")
                        nc.vector.tensor_tensor(den[:], denp[:, :, 0],
                                                denp[:, :, 1], Alu.add)
                        rden = sm.tile([128, KT], f32, tag="rden")
                        nc.vector.reciprocal(rden[:], den[:])
                        if expect_tap is not None:
                            nc.sync.dma_start(
                                expect_tap.ap()[:, b * HPC + h, :], den[:])
                        vts = []
                        for kc in range(KT):
                            vt = vsc.tile([128, DH], bf, tag="vsc")
                            nc.vector.tensor_scalar(
                                out=vt[:], in0=getv(h, b * KT + kc),
                                scalar1=rden[:, kc:kc + 1], scalar2=None,
                                op0=Alu.mult)
                            vts.append(vt)
                        for qc in range(QC):
                            pav = ps.tile([128, 512], f32, tag="ps")
                            for kc in range(KT):
                                nc.tensor.matmul(
                                    pav[:], vts[kc][:],
                                    expt[qc][:, kc, :],
                                    start=(kc == 0), stop=(kc == KT - 1))
                            nc.scalar.activation(
                                out=outT[:, h, b * S + qc * 512: b * S + (qc + 1) * 512],
                                in_=pav[:], func=Act.Copy)

            def scatter_featmajor(srcT, dst_dram):
                """srcT [128, HPC, T] bf16 -> dst_dram[j, h*DH:(h+1)*DH, :] =
                srcT[:, h, j*TSH:(j+1)*TSH]."""
                for j in range(NCORES):
                    for h in range(HPC):
                        nc.sync.dma_start(
                            dst_dram[j, h * DH:(h + 1) * DH, :],
                            srcT[:, h, j * TSH:(j + 1) * TSH])

            def outproj_ln(o_out, wpr, resid_kind, grow, brow, y_dst, xT_dst,
                           tap=None):
                """Token-sharded out-projection + residual + LN.
                resid_kind: ("xsh",) | ("y", tile, bp_row)."""
                pss = {}
                for dch in range(2):
                    for et in range(DT):
                        wrb = wrow.tile([128, 1024], bf, tag="wrow")
                        nc.sync.dma_start(
                            wrb[:], wpr[:, et, dch * 1024:(dch + 1) * 1024])
                        for tc2 in range(TSH // 128):
                            otl = ott.tile([128, 128], bf, tag="ott")
                            nc.sync.dma_start(
                                otl[:],
                                o_out[et // HPC,
                                      (et % HPC) * DH:(et % HPC + 1) * DH,
                                      tc2 * 128:(tc2 + 1) * 128])
                            for dc in range(2):
                                key = (tc2, dch * 2 + dc)
                                if et == 0:
                                    pss[key] = ps.tile([128, 512], f32, tag="ps",
                                                       name=f"op_{key}")
                                nc.tensor.matmul(
                                    pss[key][:], otl[:],
                                    wrb[:, dc * 512:(dc + 1) * 512],
                                    start=(et == 0), stop=(et == DT - 1))
                for tc2 in range(TSH // 128):
                    pre = resid.tile([128, D], f32, tag="resid")
                    if resid_kind[0] == "xsh":
                        rsh = resid.tile([128, D], f32, tag="resid")
                        nc.sync.dma_start(rsh[:], xshr[:, tc2, :])
                        for dc in range(4):
                            nc.vector.tensor_tensor(
                                pre[:, dc * 512:(dc + 1) * 512],
                                pss[(tc2, dc)][:],
                                rsh[:, dc * 512:(dc + 1) * 512], Alu.add)
                    else:
                        ybase, bprow = resid_kind[1], resid_kind[2]
                        for dc in range(4):
                            nc.vector.tensor_tensor(
                                pre[:, dc * 512:(dc + 1) * 512],
                                pss[(tc2, dc)][:],
                                ybase[:, tc2, dc * 512:(dc + 1) * 512], Alu.add)
                        for dc in range(4):
                            bpt = bc.tile([128, 512], f32, tag="bc")
                            nc.gpsimd.dma_start(
                                bpt[:], bcast_row(bprow, dc * 512, 512))
                            nc.vector.tensor_tensor(
                                pre[:, dc * 512:(dc + 1) * 512],
                                pre[:, dc * 512:(dc + 1) * 512], bpt[:], Alu.add)
                    ln_apply(pre, grow, brow, tc2, y_dst, xT_dst, None, tap)

            def ln_apply(pre, grow, brow, tc2, y_dst, xT_dst, f32_out, tap):
                stats = sm.tile([128, 4, 6], f32, tag="stats")
                for sg in range(4):
                    nc.vector.bn_stats(stats[:, sg, :],
                                       pre[:, sg * 512:(sg + 1) * 512])
                mv = sm.tile([128, 2], f32, tag="mv")
                nc.vector.bn_aggr(mv[:], stats[:])
                sd = sm.tile([128, 1], f32, tag="sd")
                nc.scalar.activation(sd[:], mv[:, 1:2], Act.Sqrt, bias=eps_sb[:])
                rstd = sm.tile([128, 1], f32, tag="rstd")
                nc.vector.reciprocal(rstd[:], sd[:])
                nmr = sm.tile([128, 1], f32, tag="nmr")
                nc.vector.tensor_tensor(nmr[:], mv[:, 0:1], rstd[:], Alu.mult)
                nc.vector.tensor_scalar_mul(nmr[:], nmr[:], -1.0)
                yf = pre
                nc.scalar.activation(yf[:], pre[:], Act.Identity,
                                     bias=nmr[:], scale=rstd[:])
                for dc in range(4):
                    sl = slice(dc * 512, (dc + 1) * 512)
                    gt = bc.tile([128, 512], f32, tag="bc")
                    nc.gpsimd.dma_start(gt[:], bcast_row(grow, dc * 512, 512))
                    nc.vector.tensor_tensor(yf[:, sl], yf[:, sl], gt[:], Alu.mult)
                    bt = bc.tile([128, 512], f32, tag="bc")
                    nc.gpsimd.dma_start(bt[:], bcast_row(brow, dc * 512, 512))
                    nc.vector.tensor_tensor(yf[:, sl], yf[:, sl], bt[:], Alu.add)
                if tap is not None:
                    nc.sync.dma_start(tap[:, tc2, :], yf[:])
                if y_dst is None:
                    # final LN: write fp32 shard out
                    nc.sync.dma_start(oshr[:, tc2, :], yf[:])
                    return
                nc.vector.tensor_copy(out=y_dst[:, tc2, :], in_=yf[:])
                # transposes -> xT_dst [128, DT, TSH] bf16
                for dt in range(DT):
                    pst = ps.tile([128, 512], f32, tag="ps")
                    nc.tensor.transpose(pst[:, :128],
                                        yf[:, dt * 128:(dt + 1) * 128], ident[:])
                    nc.vector.tensor_copy(
                        out=xT_dst[:, dt, tc2 * 128:(tc2 + 1) * 128],
                        in_=pst[:, :128])

            # ================= layer 1: self-attention =================
            qT1 = qtp.tile([128, HPC, T], bf, tag="qt")
            kT1 = ktp.tile([128, HPC, T], bf, tag="kt")
            qk_proj(xT3, wq1r, bq1_sb, qT1, nproj=2, w2dram=wk1r,
                    b2_sb=bk1_sb, dst2=kT1)

            v1 = v1p.tile([128, 16, HPC * DH], bf, tag="v1")
            for tt in range(T // 128):
                xtb1 = xblk1.tile([128, DT, 128], bf, tag="xblk1")
                nc.sync.dma_start(xtb1[:], xT3[:, :, tt * 128:(tt + 1) * 128])
                pst = ps.tile([128, 512], f32, tag="ps")
                for dt in range(DT):
                    nc.tensor.matmul(pst[:, :HPC * DH], xtb1[:, dt, :],
                                     wv1_sb[:, dt, :],
                                     start=(dt == 0), stop=(dt == DT - 1))
                nc.vector.tensor_tensor(v1[:, tt, :], pst[:, :HPC * DH],
                                        bv1_sb[:], Alu.add)

            outT1 = otp.tile([128, HPC, T], bf, tag="ot")
            attn_units(
                qT1,
                lambda h, tt: kT1[:, h, tt * 128:(tt + 1) * 128],
                lambda h, tt: v1[:, tt, h * DH:(h + 1) * DH],
                outT1)

            scatter_featmajor(outT1, o1_in)
            a2a(o1_in, o1_out)

            # cross-attn queries (only depend on encT) - overlaps the A2A
            qT2 = qtp.tile([128, HPC, T], bf, tag="qt")
            qk_proj(encT3, wq2r, bq2_sb, qT2, nproj=1)

            # out-projection 1 + residual + LN1
            y1 = yp.tile([128, TSH // 128, D], bf, tag="y")
            x1T = xtp.tile([128, DT, TSH], bf, tag="xt")
            outproj_ln(o1_out, wp1r, ("xsh",), g1r, b1r, y1, x1T,
                       tap=(taps["x1"].ap().rearrange("(o p) d -> p o d", p=128)
                            if debug_taps else None))

            # ============ layer 2: cross-attention K/V (token-sharded) ====
            for fc in range(H):
                wkb = xblk1.tile([128, DT, 128], bf, tag="xblk1")
                nc.sync.dma_start(wkb[:], wk2r[:, :, fc * 128:(fc + 1) * 128])
                pst = ps.tile([128, 512], f32, tag="ps")
                for dt in range(DT):
                    nc.tensor.matmul(pst[:, :TSH], wkb[:, dt, :], x1T[:, dt, :],
                                     start=(dt == 0), stop=(dt == DT - 1))
                ktl = kv2.tile([128, TSH], bf, tag="kv2")
                nc.scalar.activation(out=ktl[:], in_=pst[:, :TSH],
                                     func=Act.Identity, bias=bk2_sb[:, fc:fc + 1])
                nc.sync.dma_start(
                    k2_in[fc // HPC, (fc % HPC) * DH:(fc % HPC + 1) * DH, :],
                    ktl[:])
            a2a(k2_in, k2_out)

            for f4 in range(4):
                wvb = xblk.tile([128, DT, 512], bf, tag="xblk")
                nc.sync.dma_start(wvb[:], wv2r[:, :, f4 * 512:(f4 + 1) * 512])
                for tc2 in range(TSH // 128):
                    pst = ps.tile([128, 512], f32, tag="ps")
                    for dt in range(DT):
                        nc.tensor.matmul(pst[:],
                                         x1T[:, dt, tc2 * 128:(tc2 + 1) * 128],
                                         wvb[:, dt, :],
                                         start=(dt == 0), stop=(dt == DT - 1))
                    bvt = bc.tile([128, 512], f32, tag="bc")
                    nc.gpsimd.dma_start(bvt[:], bcast_row(bv2r, f4 * 512, 512))
                    vtl = kv2.tile([128, 512], bf, tag="kv2")
                    nc.vector.tensor_tensor(vtl[:], pst[:], bvt[:], Alu.add)
                    for jh in range(2):
                        nc.sync.dma_start(
                            v2_in[f4 * 2 + jh, tc2 * 128:(tc2 + 1) * 128, :],
                            vtl[:, jh * 256:(jh + 1) * 256])
            a2a(v2_in, v2_out)

            # gather my heads' K^T over all tokens
            kT2 = ktp.tile([128, HPC, T], bf, tag="kt")
            for tt in range(T // 128):
                for h in range(HPC):
                    nc.sync.dma_start(
                        kT2[:, h, tt * 128:(tt + 1) * 128],
                        k2_out[tt // 2, h * DH:(h + 1) * DH,
                               (tt % 2) * 128:(tt % 2) * 128 + 128])

            def getv2(h, tt):
                vt = vsc.tile([128, DH], bf, tag="v2l")
                nc.sync.dma_start(
                    vt[:], v2_out[tt // 2, (tt % 2) * 128:(tt % 2) * 128 + 128,
                                  h * DH:(h + 1) * DH])
                return vt[:]

            outT2 = otp.tile([128, HPC, T], bf, tag="ot")
            attn_units(
                qT2,
                lambda h, tt: kT2[:, h, tt * 128:(tt + 1) * 128],
                getv2,
                outT2)

            scatter_featmajor(outT2, o2_in)
            a2a(o2_in, o2_out)

            y2 = yp.tile([128, TSH // 128, D], bf, tag="y")
            x2T = xtp.tile([128, DT, TSH], bf, tag="xt")
            outproj_ln(o2_out, wp2r, ("y", y1, bp2r), g2r, b2r, y2, x2T,
                       tap=(taps["x2"].ap().rearrange("(o p) d -> p o d", p=128)
                            if debug_taps else None))

            # ================= FFN (token-local) =================
            ffacc = ffap.tile([128, TSH // 128, D], bf, tag="ffacc")
            for qtr in range(4):
                hTq = htp.tile([128, 16, TSH], bf, tag="ht")
                for fci in range(16):
                    fc = qtr * 16 + fci
                    wfb = xblk1.tile([128, DT, 128], bf, tag="xblk1")
                    nc.sync.dma_start(wfb[:], wf1r[:, :, fc * 128:(fc + 1) * 128])
                    pst = ps.tile([128, 512], f32, tag="ps")
                    for dt in range(DT):
                        nc.tensor.matmul(pst[:, :TSH], wfb[:, dt, :],
                                         x2T[:, dt, :],
                                         start=(dt == 0), stop=(dt == DT - 1))
                    nc.scalar.activation(out=hTq[:, fci, :], in_=pst[:, :TSH],
                                         func=Act.Relu, bias=bf1_sb[:, fc:fc + 1])
                for dch in range(2):
                    pss = {}
                    for fti in range(16):
                        ft = qtr * 16 + fti
                        wrb = wrow.tile([128, 1024], bf, tag="wrow")
                        nc.sync.dma_start(
                            wrb[:], wf2r[:, ft, dch * 1024:(dch + 1) * 1024])
                        for tc2 in range(TSH // 128):
                            for dc in range(2):
                                key = (tc2, dc)
                                if fti == 0:
                                    pss[key] = ps.tile([128, 512], f32, tag="ps",
                                                       name=f"ff_{qtr}_{dch}_{key}")
                                nc.tensor.matmul(
                                    pss[key][:],
                                    hTq[:, fti, tc2 * 128:(tc2 + 1) * 128],
                                    wrb[:, dc * 512:(dc + 1) * 512],
                                    start=(fti == 0), stop=(fti == 15))
                    for tc2 in range(TSH // 128):
                        for dc in range(2):
                            dsl = slice((dch * 2 + dc) * 512,
                                        (dch * 2 + dc + 1) * 512)
                            if qtr == 0:
                                nc.vector.tensor_copy(out=ffacc[:, tc2, dsl],
                                                      in_=pss[(tc2, dc)][:])
                            else:
                                nc.vector.tensor_tensor(
                                    ffacc[:, tc2, dsl], ffacc[:, tc2, dsl],
                                    pss[(tc2, dc)][:], Alu.add)

            for tc2 in range(TSH // 128):
                pre = resid.tile([128, D], f32, tag="resid")
                for dc in range(4):
                    sl = slice(dc * 512, (dc + 1) * 512)
                    nc.vector.tensor_tensor(pre[:, sl], ffacc[:, tc2, sl],
                                            y2[:, tc2, sl], Alu.add)
                    bft = bc.tile([128, 512], f32, tag="bc")
                    nc.gpsimd.dma_start(bft[:], bcast_row(bf2r, dc * 512, 512))
                    nc.vector.tensor_tensor(pre[:, sl], pre[:, sl], bft[:],
                                            Alu.add)
                ln_apply(pre, g3r, b3r, tc2, None, None, None, None)

            if debug_taps:
                tmp = pool.tile([128, HPC, T], f32)
                nc.vector.tensor_copy(out=tmp[:], in_=qT1[:])
                nc.sync.dma_start(taps["qT1"].ap()[:], tmp[:])

    nc.compile()
    return nc


def _prep_inputs(inputs):
    """Host-side shard/transpose/cast. Returns list of per-core in_maps."""
    g = {k: np.asarray(v, np.float32) for k, v in inputs.items()}
    s = 1.0 / np.sqrt(np.float32(DH))
    xf = g["x"].reshape(T, D)
    encf = g["encoder_output"].reshape(T, D)
    xT = np.ascontiguousarray(xf.T).astype(BF16)
    encT = np.ascontiguousarray(encf.T).astype(BF16)
    shared = {
        "xT": xT, "encT": encT,
        "wk2a": np.ascontiguousarray(
            g["wk2"].transpose(1, 0, 2).reshape(D, D)).astype(BF16),
        "bk2a": np.ascontiguousarray(g["bk2"].T),
        "wv2a": np.ascontiguousarray(
            g["wv2"].transpose(1, 0, 2).reshape(D, D)).astype(BF16),
        "bv2r": g["bv2"].reshape(1, D).copy(),
        "wp1": g["wp1"].astype(BF16),
        "wp2": g["wp2"].astype(BF16),
        "bp2r": g["bp2"].reshape(1, D).copy(),
        "wf1": g["w_ff1"].astype(BF16),
        "bf1c": np.ascontiguousarray(g["b_ff1"].reshape(FT, DH).T),
        "wf2": g["w_ff2"].astype(BF16),
        "bf2r": g["b_ff2"].reshape(1, D).copy(),
        "g1r": g["ln1_g"].reshape(1, D).copy(),
        "b1r": g["ln1_b"].reshape(1, D).copy(),
        "g2r": g["ln2_g"].reshape(1, D).copy(),
        "b2r": g["ln2_b"].reshape(1, D).copy(),
        "g3r": g["ln3_g"].reshape(1, D).copy(),
        "b3r": g["ln3_b"].reshape(1, D).copy(),
    }
    in_maps = []
    for c in range(NCORES):
        hs = slice(HPC * c, HPC * (c + 1))
        m = dict(shared)
        m["xsh"] = xf[TSH * c: TSH * (c + 1)] + g["bp1"][None, :]
        m["wq1"] = np.ascontiguousarray(
            g["wq1"][hs].transpose(1, 0, 2).reshape(D, HPC * DH) * s).astype(BF16)
        m["wk1"] = np.ascontiguousarray(
            g["wk1"][hs].transpose(1, 0, 2).reshape(D, HPC * DH)).astype(BF16)
        m["wv1"] = np.ascontiguousarray(
            g["wv1"][hs].transpose(1, 0, 2).reshape(D, HPC * DH)).astype(BF16)
        m["bq1"] = np.ascontiguousarray(g["bq1"][hs].T * s)
        m["bk1"] = np.ascontiguousarray(g["bk1"][hs].T)
        m["bv1r"] = g["bv1"][hs].reshape(1, HPC * DH).copy()
        m["wq2"] = np.ascontiguousarray(
            g["wq2"][hs].transpose(1, 0, 2).reshape(D, HPC * DH) * s).astype(BF16)
        m["bq2"] = np.ascontiguousarray(g["bq2"][hs].T * s)
        in_maps.append(m)
    return in_maps


def kernel(**inputs):
    from concourse import bass_utils
    if "prog" not in _CACHE:
        _CACHE["prog"] = _build()
    nc = _CACHE["prog"]
    in_maps = _prep_inputs(inputs)
    res = bass_utils.run_bass_kernel_spmd(
        nc, in_maps, core_ids=list(range(NCORES)))
    _CACHE["last_result"] = res
    out = np.concatenate([res.results[c]["osh"] for c in range(NCORES)], axis=0)
    return out.reshape(B, S, D).astype(np.float32)


# revision 10
# speedup vs baseline: 1.0511x; 1.0511x over previous
"""Self-contained Trainium2 (8-core) kernel for nn_DecoderBlock_82660940579418.

Decoder block: self-attn -> LN -> cross-attn -> LN -> FFN -> LN, with the
reference's softmax over the *query* axis.

Sharding: attention heads (2 per core) for QKV/scores/AV; tokens (256 per
core) for out-projections, LayerNorms and the FFN. Cross-core traffic is
four 1 MB AllToAlls (bf16). Compute dtype bf16 on TensorE, fp32 PSUM,
fp32 residual stream and LayerNorm.
"""

import os
import sys

for _p in ("/opt/trn_rl_repo", "/root/.axon_site/_ro/trn_rl_repo"):
    if os.path.isdir(_p) and _p not in sys.path:
        sys.path.append(_p)

import numpy as np
import ml_dtypes

B, S, D, H, DH, DFF = 2, 1024, 2048, 16, 128, 8192
NCORES = 8
T = B * S            # 2048 tokens, index t = b*S + s
TSH = T // NCORES    # 256 tokens per core
HPC = H // NCORES    # 2 heads per core
DT = D // 128        # 16 feature tiles
FT = DFF // 128      # 64 ffn-hidden tiles
BF16 = ml_dtypes.bfloat16

_CACHE = {}


def _build(debug_taps=False):
    import concourse.bass as bass
    import concourse.mybir as mybir
    import concourse.tile as tile
    from concourse import bacc
    from concourse.masks import make_identity

    f32 = mybir.dt.float32
    bf = mybir.dt.bfloat16
    Act = mybir.ActivationFunctionType
    Alu = mybir.AluOpType

    nc = bacc.Bacc("TRN2", target_bir_lowering=False, debug=False,
                   num_devices=NCORES)

    def din(name, shape, dt):
        return nc.dram_tensor(name, shape, dt, kind="ExternalInput")

    # ---- external inputs (per core) ----
    xT = din("xT", [D, T], bf)           # x^T, feature-major, full
    encT = din("encT", [D, T], bf)       # encoder_output^T, full
    xsh = din("xsh", [TSH, D], f32)      # my token shard of x, + bp1 folded
    wq1 = din("wq1", [D, HPC * DH], bf)  # my heads, pre-scaled by DH^-0.5
    wk1 = din("wk1", [D, HPC * DH], bf)
    wv1 = din("wv1", [D, HPC * DH], bf)
    bq1 = din("bq1", [DH, HPC], f32)     # pre-scaled
    bk1 = din("bk1", [DH, HPC], f32)
    bv1r = din("bv1r", [1, HPC * DH], f32)
    wp1 = din("wp1", [D, D], bf)
    wq2 = din("wq2", [D, HPC * DH], bf)  # pre-scaled
    bq2 = din("bq2", [DH, HPC], f32)
    wk2a = din("wk2a", [D, D], bf)       # all heads
    bk2a = din("bk2a", [DH, H], f32)
    wv2a = din("wv2a", [D, D], bf)
    bv2r = din("bv2r", [1, D], f32)
    wp2 = din("wp2", [D, D], bf)
    bp2r = din("bp2r", [1, D], f32)
    wf1 = din("wf1", [D, DFF], bf)
    bf1c = din("bf1c", [DH, FT], f32)
    wf2 = din("wf2", [DFF, D], bf)
    bf2r = din("bf2r", [1, D], f32)
    g1r = din("g1r", [1, D], f32)
    b1r = din("b1r", [1, D], f32)
    g2r = din("g2r", [1, D], f32)
    b2r = din("b2r", [1, D], f32)
    g3r = din("g3r", [1, D], f32)
    b3r = din("b3r", [1, D], f32)

    osh = nc.dram_tensor("osh", [TSH, D], f32, kind="ExternalOutput")

    taps = {}
    if debug_taps:
        taps["qT1"] = nc.dram_tensor("tap_qT1", [DH, HPC, T], f32, kind="ExternalOutput")
        taps["den1"] = nc.dram_tensor("tap_den1", [DH, 4, 8], f32, kind="ExternalOutput")
        taps["outT1"] = nc.dram_tensor("tap_outT1", [DH, HPC, T], f32, kind="ExternalOutput")
        taps["x1"] = nc.dram_tensor("tap_x1", [TSH, D], f32, kind="ExternalOutput")
        taps["x2"] = nc.dram_tensor("tap_x2", [TSH, D], f32, kind="ExternalOutput")
        taps["kT2"] = nc.dram_tensor("tap_kT2", [DH, HPC, T], f32, kind="ExternalOutput")
        taps["v2"] = nc.dram_tensor("tap_v2", [DH, 16, HPC * DH], f32, kind="ExternalOutput")

    # rearranged DRAM views: [128, tile, free]
    xT3 = xT.ap().rearrange("(o p) t -> p o t", p=128)
    encT3 = encT.ap().rearrange("(o p) t -> p o t", p=128)
    wq1r = wq1.ap().rearrange("(o p) e -> p o e", p=128)
    wk1r = wk1.ap().rearrange("(o p) e -> p o e", p=128)
    wv1r = wv1.ap().rearrange("(o p) e -> p o e", p=128)
    wq2r = wq2.ap().rearrange("(o p) e -> p o e", p=128)
    wk2r = wk2a.ap().rearrange("(o p) f -> p o f", p=128)
    wv2r = wv2a.ap().rearrange("(o p) f -> p o f", p=128)
    wp1r = wp1.ap().rearrange("(o p) d -> p o d", p=128)
    wp2r = wp2.ap().rearrange("(o p) d -> p o d", p=128)
    wf1r = wf1.ap().rearrange("(o p) f -> p o f", p=128)
    wf2r = wf2.ap().rearrange("(o p) d -> p o d", p=128)
    xshr = xsh.ap().rearrange("(o p) d -> p o d", p=128)
    oshr = osh.ap().rearrange("(o p) d -> p o d", p=128)

    def bcast_row(row_t, lo, n, p=128):
        ap = row_t.ap()
        st = ap.ap[-1][0]
        return bass.AP(tensor=ap.tensor, offset=ap.offset + lo * st,
                       ap=[[0, p], [st, n]])

    with tile.TileContext(nc) as tc:
        import contextlib
        ctx = contextlib.ExitStack()
        with ctx:
            pool = ctx.enter_context(tc.tile_pool(name="persist", bufs=1))
            qtp = ctx.enter_context(tc.tile_pool(name="qtp", bufs=1))
            ktp = ctx.enter_context(tc.tile_pool(name="ktp", bufs=1))
            v1p = ctx.enter_context(tc.tile_pool(name="v1p", bufs=1))
            xtp = ctx.enter_context(tc.tile_pool(name="xtp", bufs=1))
            yp = ctx.enter_context(tc.tile_pool(name="yp", bufs=2))
            expp = ctx.enter_context(tc.tile_pool(name="expp", bufs=4))
            htp = ctx.enter_context(tc.tile_pool(name="htp", bufs=2))
            xblk = ctx.enter_context(tc.tile_pool(name="xblk", bufs=2))
            xblk1 = ctx.enter_context(tc.tile_pool(name="xblk1", bufs=3))
            wrow = ctx.enter_context(tc.tile_pool(name="wrow", bufs=2))
            ott = ctx.enter_context(tc.tile_pool(name="ott", bufs=4))
            vsc = ctx.enter_context(tc.tile_pool(name="vsc", bufs=8))
            kv2 = ctx.enter_context(tc.tile_pool(name="kv2", bufs=3))
            bc = ctx.enter_context(tc.tile_pool(name="bc", bufs=2))
            sm = ctx.enter_context(tc.tile_pool(name="sm", bufs=6))
            resid = ctx.enter_context(tc.tile_pool(name="resid", bufs=2))
            ffap = ctx.enter_context(tc.tile_pool(name="ffap", bufs=1))
            ps = ctx.enter_context(tc.tile_pool(name="ps", bufs=8, space="PSUM"))
            dram = ctx.enter_context(tc.tile_pool(name="dram", bufs=1, space="DRAM"))

            # ---- constants / persistent weights in SBUF ----
            ident = pool.tile([128, 128], f32)
            make_identity(nc, ident[:])
            eps_sb = pool.tile([128, 1], f32)
            nc.vector.memset(eps_sb[:], 1e-5)

            wv1_sb = pool.tile([128, DT, HPC * DH], bf)
            nc.sync.dma_start(wv1_sb[:], wv1r)
            bq1_sb = pool.tile([128, HPC], f32)
            nc.sync.dma_start(bq1_sb[:], bq1.ap()[:])
            bk1_sb = pool.tile([128, HPC], f32)
            nc.sync.dma_start(bk1_sb[:], bk1.ap()[:])
            bq2_sb = pool.tile([128, HPC], f32)
            nc.sync.dma_start(bq2_sb[:], bq2.ap()[:])
            bk2_sb = pool.tile([128, H], f32)
            nc.sync.dma_start(bk2_sb[:], bk2a.ap()[:])
            bf1_sb = pool.tile([128, FT], f32)
            nc.sync.dma_start(bf1_sb[:], bf1c.ap()[:])
            bv1_sb = pool.tile([128, HPC * DH], f32)
            nc.gpsimd.dma_start(bv1_sb[:], bcast_row(bv1r, 0, HPC * DH))

            # ---- DRAM bounce buffers for the 4 AllToAlls ----
            o1_in = dram.tile([NCORES, HPC * DH, TSH], bf)
            o1_out = dram.tile([NCORES, HPC * DH, TSH], bf)
            k2_in = dram.tile([NCORES, HPC * DH, TSH], bf)
            k2_out = dram.tile([NCORES, HPC * DH, TSH], bf)
            v2_in = dram.tile([NCORES, TSH, HPC * DH], bf)
            v2_out = dram.tile([NCORES, TSH, HPC * DH], bf)
            o2_in = dram.tile([NCORES, HPC * DH, TSH], bf)
            o2_out = dram.tile([NCORES, HPC * DH, TSH], bf)
            rg = [list(range(NCORES))]

            def a2a(src, dst):
                nc.gpsimd.collective_compute(
                    "AllToAll", Alu.bypass, replica_groups=rg,
                    ins=[src.opt()], outs=[dst.opt()])

            # ================= helpers =================
            def qk_proj(src3, wdram, b_sb, dst, nproj=2, w2dram=None, b2_sb=None,
                        dst2=None):
                """Feature-major projections for my heads over all tokens.
                dst[:, h, t] = sum_d w[d, h*DH+e] * src[d, t] + b[e, h]."""
                for tc4 in range(T // 512):
                    xtb = xblk.tile([128, DT, 512], bf, tag="xblk")
                    nc.sync.dma_start(xtb[:], src3[:, :, tc4 * 512:(tc4 + 1) * 512])
                    wt1 = xblk1.tile([128, DT, HPC * DH], bf, tag="xblk1")
                    nc.gpsimd.dma_start(wt1[:], wdram)
                    plist = [(wt1, b_sb, dst)]
                    if nproj == 2:
                        wt2 = xblk1.tile([128, DT, HPC * DH], bf, tag="xblk1")
                        nc.gpsimd.dma_start(wt2[:], w2dram)
                        plist.append((wt2, b2_sb, dst2))
                    for h in range(HPC):
                        for (wsb, bsb, dd) in plist:
                            pst = ps.tile([128, 512], f32, tag="ps")
                            for dt in range(DT):
                                nc.tensor.matmul(
                                    pst[:], wsb[:, dt, h * DH:(h + 1) * DH],
                                    xtb[:, dt, :],
                                    start=(dt == 0), stop=(dt == DT - 1))
                            nc.vector.tensor_scalar(
                                out=dd[:, h, tc4 * 512:(tc4 + 1) * 512],
                                in0=pst[:], scalar1=bsb[:, h:h + 1],
                                scalar2=None, op0=Alu.add)

            def attn_units(qT, kTs, getv, o_in):
                """Per (b, h): scores^T=[k,q] -> exp (ACT) -> denom (DVE
                reduce) -> scale V rows -> AV -> outT[:, h, b*S+q].
                Units are software-pipelined: unit u+1's scores are issued
                before unit u's AV so the PE stays busy during exp."""
                KT = S // 128   # 8 k tiles per batch
                QC = S // 512   # 2 q chunks per batch
                units = [(b, h) for b in range(B) for h in range(HPC)]

                def scores_phase(b, h):
                    expt = [expp.tile([128, KT, 512], bf, tag="expt",
                                      name=f"exp_{b}_{h}_{qc}")
                            for qc in range(QC)]
                    dred = sm.tile([128, QC, KT], f32, tag="dred")
                    for qc in range(QC):
                        for kc in range(KT):
                            pst = ps.tile([128, 512], f32, tag="ps")
                            nc.tensor.matmul(
                                pst[:],
                                kTs(h, b * KT + kc),
                                qT[:, h, b * S + qc * 512: b * S + (qc + 1) * 512],
                                start=True, stop=True)
                            nc.scalar.activation(
                                out=expt[qc][:, kc, :],
                                in_=pst[:], func=Act.Exp)
                        nc.vector.tensor_reduce(
                            out=dred[:, qc, :], in_=expt[qc][:],
                            axis=mybir.AxisListType.X, op=Alu.add)
                    return expt, dred

                def av_phase(b, h, expt, dred):
                    den = sm.tile([128, KT], f32, tag="den")
                    nc.vector.tensor_tensor(den[:], dred[:, 0, :],
                                            dred[:, 1, :], Alu.add)
                    rden = sm.tile([128, KT], f32, tag="rden")
                    nc.vector.reciprocal(rden[:], den[:])
                    vts = []
                    for kc in range(KT):
                        vt = vsc.tile([128, DH], bf, tag="vsc")
                        nc.vector.tensor_scalar(
                            out=vt[:], in0=getv(h, b * KT + kc),
                            scalar1=rden[:, kc:kc + 1], scalar2=None,
                            op0=Alu.mult)
                        vts.append(vt)
                    for qc in range(QC):
                        pav = ps.tile([128, 512], f32, tag="ps")
                        for kc in range(KT):
                            nc.tensor.matmul(
                                pav[:], vts[kc][:],
                                expt[qc][:, kc, :],
                                start=(kc == 0), stop=(kc == KT - 1))
                        ot = kv2.tile([128, 512], bf, tag="kv2")
                        nc.scalar.activation(out=ot[:], in_=pav[:],
                                             func=Act.Copy)
                        j0 = (b * S + qc * 512) // TSH
                        for jj in range(2):
                            nc.sync.dma_start(
                                o_in[j0 + jj, h * DH:(h + 1) * DH, :],
                                ot[:, jj * 256:(jj + 1) * 256])

                pending = None
                for (b, h) in units:
                    cur = (b, h, *scores_phase(b, h))
                    if pending is not None:
                        pb, ph, pe_, pd = pending
                        av_phase(pb, ph, pe_, pd)
                    pending = cur
                pb, ph, pe_, pd = pending
                av_phase(pb, ph, pe_, pd)

            def outproj_ln(o_out, wpr, resid_kind, grow, brow, y_dst, xT_dst,
                           tap=None):
                """Token-sharded out-projection + residual + LN.
                resid_kind: ("xsh",) | ("y", tile, bp_row)."""
                pss = {}
                for dch in range(2):
                    for et in range(DT):
                        wrb = wrow.tile([128, 1024], bf, tag="wrow")
                        nc.gpsimd.dma_start(
                            wrb[:], wpr[:, et, dch * 1024:(dch + 1) * 1024])
                        for tc2 in range(TSH // 128):
                            otl = ott.tile([128, 128], bf, tag="ott")
                            nc.gpsimd.dma_start(
                                otl[:],
                                o_out[et // HPC,
                                      (et % HPC) * DH:(et % HPC + 1) * DH,
                                      tc2 * 128:(tc2 + 1) * 128])
                            for dc in range(2):
                                key = (tc2, dch * 2 + dc)
                                if et == 0:
                                    pss[key] = ps.tile([128, 512], f32, tag="ps",
                                                       name=f"op_{key}")
                                nc.tensor.matmul(
                                    pss[key][:], otl[:],
                                    wrb[:, dc * 512:(dc + 1) * 512],
                                    start=(et == 0), stop=(et == DT - 1))
                for tc2 in range(TSH // 128):
                    pre = resid.tile([128, D], f32, tag="resid")
                    if resid_kind[0] == "xsh":
                        rsh = resid.tile([128, D], f32, tag="resid")
                        nc.sync.dma_start(rsh[:], xshr[:, tc2, :])
                        for dc in range(4):
                            nc.vector.tensor_tensor(
                                pre[:, dc * 512:(dc + 1) * 512],
                                pss[(tc2, dc)][:],
                                rsh[:, dc * 512:(dc + 1) * 512], Alu.add)
                    else:
                        ybase, bprow = resid_kind[1], resid_kind[2]
                        for dc in range(4):
                            nc.vector.tensor_tensor(
                                pre[:, dc * 512:(dc + 1) * 512],
                                pss[(tc2, dc)][:],
                                ybase[:, tc2, dc * 512:(dc + 1) * 512], Alu.add)
                        for dc in range(4):
                            bpt = bc.tile([128, 512], f32, tag="bc")
                            nc.gpsimd.dma_start(
                                bpt[:], bcast_row(bprow, dc * 512, 512))
                            nc.vector.tensor_tensor(
                                pre[:, dc * 512:(dc + 1) * 512],
                                pre[:, dc * 512:(dc + 1) * 512], bpt[:], Alu.add)
                    ln_apply(pre, grow, brow, tc2, y_dst, xT_dst, None, tap)

            def ln_apply(pre, grow, brow, tc2, y_dst, xT_dst, f32_out, tap):
                stats = sm.tile([128, 4, 6], f32, tag="stats")
                for sg in range(4):
                    nc.vector.bn_stats(stats[:, sg, :],
                                       pre[:, sg * 512:(sg + 1) * 512])
                mv = sm.tile([128, 2], f32, tag="mv")
                nc.vector.bn_aggr(mv[:], stats[:])
                sd = sm.tile([128, 1], f32, tag="sd")
                nc.scalar.activation(sd[:], mv[:, 1:2], Act.Sqrt, bias=eps_sb[:])
                rstd = sm.tile([128, 1], f32, tag="rstd")
                nc.vector.reciprocal(rstd[:], sd[:])
                nmr = sm.tile([128, 1], f32, tag="nmr")
                nc.vector.tensor_tensor(nmr[:], mv[:, 0:1], rstd[:], Alu.mult)
                nc.vector.tensor_scalar_mul(nmr[:], nmr[:], -1.0)
                yf = pre
                nc.scalar.activation(yf[:], pre[:], Act.Identity,
                                     bias=nmr[:], scale=rstd[:])
                for dc in range(4):
                    sl = slice(dc * 512, (dc + 1) * 512)
                    gt = bc.tile([128, 512], f32, tag="bc")
                    nc.gpsimd.dma_start(gt[:], bcast_row(grow, dc * 512, 512))
                    nc.vector.tensor_tensor(yf[:, sl], yf[:, sl], gt[:], Alu.mult)
                    bt = bc.tile([128, 512], f32, tag="bc")
                    nc.gpsimd.dma_start(bt[:], bcast_row(brow, dc * 512, 512))
                    nc.vector.tensor_tensor(yf[:, sl], yf[:, sl], bt[:], Alu.add)
                if tap is not None:
                    nc.sync.dma_start(tap[:, tc2, :], yf[:])
                if y_dst is None:
                    # final LN: write fp32 shard out
                    nc.sync.dma_start(oshr[:, tc2, :], yf[:])
                    return
                nc.vector.tensor_copy(out=y_dst[:, tc2, :], in_=yf[:])
                # transposes -> xT_dst [128, DT, TSH] bf16
                for dt in range(DT):
                    pst = ps.tile([128, 512], f32, tag="ps")
                    nc.tensor.transpose(pst[:, :128],
                                        yf[:, dt * 128:(dt + 1) * 128], ident[:])
                    nc.vector.tensor_copy(
                        out=xT_dst[:, dt, tc2 * 128:(tc2 + 1) * 128],
                        in_=pst[:, :128])

            # ================= layer 1: self-attention =================
            qT1 = qtp.tile([128, HPC, T], bf, tag="qt")
            kT1 = ktp.tile([128, HPC, T], bf, tag="kt")
            qk_proj(xT3, wq1r, bq1_sb, qT1, nproj=2, w2dram=wk1r,
                    b2_sb=bk1_sb, dst2=kT1)

            v1 = v1p.tile([128, 16, HPC * DH], bf, tag="v1")
            for tt in range(T // 128):
                xtb1 = xblk1.tile([128, DT, 128], bf, tag="xblk1")
                nc.sync.dma_start(xtb1[:], xT3[:, :, tt * 128:(tt + 1) * 128])
                pst = ps.tile([128, 512], f32, tag="ps")
                for dt in range(DT):
                    nc.tensor.matmul(pst[:, :HPC * DH], xtb1[:, dt, :],
                                     wv1_sb[:, dt, :],
                                     start=(dt == 0), stop=(dt == DT - 1))
                nc.vector.tensor_tensor(v1[:, tt, :], pst[:, :HPC * DH],
                                        bv1_sb[:], Alu.add)

            attn_units(
                qT1,
                lambda h, tt: kT1[:, h, tt * 128:(tt + 1) * 128],
                lambda h, tt: v1[:, tt, h * DH:(h + 1) * DH],
                o1_in)
            a2a(o1_in, o1_out)

            # cross-attn queries (only depend on encT) - overlaps the A2A
            qT2 = qtp.tile([128, HPC, T], bf, tag="qt")
            qk_proj(encT3, wq2r, bq2_sb, qT2, nproj=1)

            # out-projection 1 + residual + LN1
            y1 = yp.tile([128, TSH // 128, D], bf, tag="y")
            x1T = xtp.tile([128, DT, TSH], bf, tag="xt")
            outproj_ln(o1_out, wp1r, ("xsh",), g1r, b1r, y1, x1T,
                       tap=(taps["x1"].ap().rearrange("(o p) d -> p o d", p=128)
                            if debug_taps else None))

            # ============ layer 2: cross-attention K/V (token-sharded) ====
            for fc in range(H):
                wkb = xblk1.tile([128, DT, 128], bf, tag="xblk1")
                nc.gpsimd.dma_start(wkb[:], wk2r[:, :, fc * 128:(fc + 1) * 128])
                pst = ps.tile([128, 512], f32, tag="ps")
                for dt in range(DT):
                    nc.tensor.matmul(pst[:, :TSH], wkb[:, dt, :], x1T[:, dt, :],
                                     start=(dt == 0), stop=(dt == DT - 1))
                ktl = kv2.tile([128, TSH], bf, tag="kv2")
                nc.vector.tensor_scalar(out=ktl[:], in0=pst[:, :TSH],
                                        scalar1=bk2_sb[:, fc:fc + 1],
                                        scalar2=None, op0=Alu.add)
                nc.sync.dma_start(
                    k2_in[fc // HPC, (fc % HPC) * DH:(fc % HPC + 1) * DH, :],
                    ktl[:])
            a2a(k2_in, k2_out)

            for f4 in range(4):
                wvb = xblk.tile([128, DT, 512], bf, tag="xblk")
                nc.gpsimd.dma_start(wvb[:], wv2r[:, :, f4 * 512:(f4 + 1) * 512])
                for tc2 in range(TSH // 128):
                    pst = ps.tile([128, 512], f32, tag="ps")
                    for dt in range(DT):
                        nc.tensor.matmul(pst[:],
                                         x1T[:, dt, tc2 * 128:(tc2 + 1) * 128],
                                         wvb[:, dt, :],
                                         start=(dt == 0), stop=(dt == DT - 1))
                    bvt = bc.tile([128, 512], f32, tag="bc")
                    nc.gpsimd.dma_start(bvt[:], bcast_row(bv2r, f4 * 512, 512))
                    vtl = kv2.tile([128, 512], bf, tag="kv2")
                    nc.vector.tensor_tensor(vtl[:], pst[:], bvt[:], Alu.add)
                    for jh in range(2):
                        nc.sync.dma_start(
                            v2_in[f4 * 2 + jh, tc2 * 128:(tc2 + 1) * 128, :],
                            vtl[:, jh * 256:(jh + 1) * 256])
            a2a(v2_in, v2_out)

            # gather my heads' K^T over all tokens
            kT2 = ktp.tile([128, HPC, T], bf, tag="kt")
            for j in range(NCORES):
                for h in range(HPC):
                    nc.sync.dma_start(
                        kT2[:, h, j * TSH:(j + 1) * TSH],
                        k2_out[j, h * DH:(h + 1) * DH, :])

            def getv2(h, tt):
                vt = vsc.tile([128, DH], bf, tag="v2l")
                nc.sync.dma_start(
                    vt[:], v2_out[tt // 2, (tt % 2) * 128:(tt % 2) * 128 + 128,
                                  h * DH:(h + 1) * DH])
                return vt[:]

            attn_units(
                qT2,
                lambda h, tt: kT2[:, h, tt * 128:(tt + 1) * 128],
                getv2,
                o2_in)
            a2a(o2_in, o2_out)

            y2 = yp.tile([128, TSH // 128, D], bf, tag="y")
            x2T = xtp.tile([128, DT, TSH], bf, tag="xt")
            outproj_ln(o2_out, wp2r, ("y", y1, bp2r), g2r, b2r, y2, x2T,
                       tap=(taps["x2"].ap().rearrange("(o p) d -> p o d", p=128)
                            if debug_taps else None))

            # ================= FFN (token-local) =================
            ffacc = ffap.tile([128, TSH // 128, D], bf, tag="ffacc")
            for qtr in range(4):
                hTq = htp.tile([128, 16, TSH], bf, tag="ht")
                for fci in range(16):
                    fc = qtr * 16 + fci
                    wfb = xblk1.tile([128, DT, 128], bf, tag="xblk1")
                    nc.gpsimd.dma_start(wfb[:], wf1r[:, :, fc * 128:(fc + 1) * 128])
                    pst = ps.tile([128, 512], f32, tag="ps")
                    for dt in range(DT):
                        nc.tensor.matmul(pst[:, :TSH], wfb[:, dt, :],
                                         x2T[:, dt, :],
                                         start=(dt == 0), stop=(dt == DT - 1))
                    nc.scalar.activation(out=hTq[:, fci, :], in_=pst[:, :TSH],
                                         func=Act.Relu, bias=bf1_sb[:, fc:fc + 1])
                for dch in range(2):
                    pss = {}
                    for fti in range(16):
                        ft = qtr * 16 + fti
                        wrb = wrow.tile([128, 1024], bf, tag="wrow")
                        nc.gpsimd.dma_start(
                            wrb[:], wf2r[:, ft, dch * 1024:(dch + 1) * 1024])
                        for tc2 in range(TSH // 128):
                            for dc in range(2):
                                key = (tc2, dc)
                                if fti == 0:
                                    pss[key] = ps.tile([128, 512], f32, tag="ps",
                                                       name=f"ff_{qtr}_{dch}_{key}")
                                nc.tensor.matmul(
                                    pss[key][:],
                                    hTq[:, fti, tc2 * 128:(tc2 + 1) * 128],
                                    wrb[:, dc * 512:(dc + 1) * 512],
                                    start=(fti == 0), stop=(fti == 15))
                    for tc2 in range(TSH // 128):
                        for dc in range(2):
                            dsl = slice((dch * 2 + dc) * 512,
                                        (dch * 2 + dc + 1) * 512)
                            if qtr == 0:
                                nc.vector.tensor_copy(out=ffacc[:, tc2, dsl],
                                                      in_=pss[(tc2, dc)][:])
                            else:
                                nc.vector.tensor_tensor(
                                    ffacc[:, tc2, dsl], ffacc[:, tc2, dsl],
                                    pss[(tc2, dc)][:], Alu.add)

            for tc2 in range(TSH // 128):
                pre = resid.tile([128, D], f32, tag="resid")
                for dc in range(4):
                    sl = slice(dc * 512, (dc + 1) * 512)
                    nc.vector.tensor_tensor(pre[:, sl], ffacc[:, tc2, sl],
                                            y2[:, tc2, sl], Alu.add)
                    bft = bc.tile([128, 512], f32, tag="bc")
                    nc.gpsimd.dma_start(bft[:], bcast_row(bf2r, dc * 512, 512))
                    nc.vector.tensor_tensor(pre[:, sl], pre[:, sl], bft[:],
                                            Alu.add)
                ln_apply(pre, g3r, b3r, tc2, None, None, None, None)

            if debug_taps:
                tmp = pool.tile([128, HPC, T], f32)
                nc.vector.tensor_copy(out=tmp[:], in_=qT1[:])
                nc.sync.dma_start(taps["qT1"].ap()[:], tmp[:])

    nc.compile()
    return nc


def _prep_inputs(inputs):
    """Host-side shard/transpose/cast. Returns list of per-core in_maps."""
    g = {k: np.asarray(v, np.float32) for k, v in inputs.items()}
    s = 1.0 / np.sqrt(np.float32(DH))
    xf = g["x"].reshape(T, D)
    encf = g["encoder_output"].reshape(T, D)
    xT = np.ascontiguousarray(xf.T).astype(BF16)
    encT = np.ascontiguousarray(encf.T).astype(BF16)
    shared = {
        "xT": xT, "encT": encT,
        "wk2a": np.ascontiguousarray(
            g["wk2"].transpose(1, 0, 2).reshape(D, D)).astype(BF16),
        "bk2a": np.ascontiguousarray(g["bk2"].T),
        "wv2a": np.ascontiguousarray(
            g["wv2"].transpose(1, 0, 2).reshape(D, D)).astype(BF16),
        "bv2r": g["bv2"].reshape(1, D).copy(),
        "wp1": g["wp1"].astype(BF16),
        "wp2": g["wp2"].astype(BF16),
        "bp2r": g["bp2"].reshape(1, D).copy(),
        "wf1": g["w_ff1"].astype(BF16),
        "bf1c": np.ascontiguousarray(g["b_ff1"].reshape(FT, DH).T),
        "wf2": g["w_ff2"].astype(BF16),
        "bf2r": g["b_ff2"].reshape(1, D).copy(),
        "g1r": g["ln1_g"].reshape(1, D).copy(),
        "b1r": g["ln1_b"].reshape(1, D).copy(),
        "g2r": g["ln2_g"].reshape(1, D).copy(),
        "b2r": g["ln2_b"].reshape(1, D).copy(),
        "g3r": g["ln3_g"].reshape(1, D).copy(),
        "b3r": g["ln3_b"].reshape(1, D).copy(),
    }
    in_maps = []
    for c in range(NCORES):
        hs = slice(HPC * c, HPC * (c + 1))
        m = dict(shared)
        m["xsh"] = xf[TSH * c: TSH * (c + 1)] + g["bp1"][None, :]
        m["wq1"] = np.ascontiguousarray(
            g["wq1"][hs].transpose(1, 0, 2).reshape(D, HPC * DH) * s).astype(BF16)
        m["wk1"] = np.ascontiguousarray(
            g["wk1"][hs].transpose(1, 0, 2).reshape(D, HPC * DH)).astype(BF16)
        m["wv1"] = np.ascontiguousarray(
            g["wv1"][hs].transpose(1, 0, 2).reshape(D, HPC * DH)).astype(BF16)
        m["bq1"] = np.ascontiguousarray(g["bq1"][hs].T * s)
        m["bk1"] = np.ascontiguousarray(g["bk1"][hs].T)
        m["bv1r"] = g["bv1"][hs].reshape(1, HPC * DH).copy()
        m["wq2"] = np.ascontiguousarray(
            g["wq2"][hs].transpose(1, 0, 2).reshape(D, HPC * DH) * s).astype(BF16)
        m["bq2"] = np.ascontiguousarray(g["bq2"][hs].T * s)
        in_maps.append(m)
    return in_maps


def kernel(**inputs):
    from concourse import bass_utils
    if "prog" not in _CACHE:
        _CACHE["prog"] = _build()
    nc = _CACHE["prog"]
    in_maps = _prep_inputs(inputs)
    res = bass_utils.run_bass_kernel_spmd(
        nc, in_maps, core_ids=list(range(NCORES)))
    _CACHE["last_result"] = res
    out = np.concatenate([res.results[c]["osh"] for c in range(NCORES)], axis=0)
    return out.reshape(B, S, D).astype(np.float32)


# revision 15
# speedup vs baseline: 1.0935x; 1.0403x over previous
"""Self-contained Trainium2 (8-core) kernel for nn_DecoderBlock_82660940579418.

Decoder block: self-attn -> LN -> cross-attn -> LN -> FFN -> LN, with the
reference's softmax over the *query* axis.

Sharding: attention heads (2 per core) for QKV/scores/AV; tokens (256 per
core) for out-projections, LayerNorms and the FFN. Cross-core traffic is
four 1 MB AllToAlls (bf16). Compute dtype bf16 on TensorE, fp32 PSUM,
fp32 residual stream and LayerNorm.
"""

import os
import sys

for _p in ("/opt/trn_rl_repo", "/root/.axon_site/_ro/trn_rl_repo"):
    if os.path.isdir(_p) and _p not in sys.path:
        sys.path.append(_p)

import numpy as np
import ml_dtypes

B, S, D, H, DH, DFF = 2, 1024, 2048, 16, 128, 8192
NCORES = 8
T = B * S            # 2048 tokens, index t = b*S + s
TSH = T // NCORES    # 256 tokens per core
HPC = H // NCORES    # 2 heads per core
DT = D // 128        # 16 feature tiles
FT = DFF // 128      # 64 ffn-hidden tiles
BF16 = ml_dtypes.bfloat16

_CACHE = {}


def _build(debug_taps=False):
    import concourse.bass as bass
    import concourse.mybir as mybir
    import concourse.tile as tile
    from concourse import bacc
    from concourse.masks import make_identity

    f32 = mybir.dt.float32
    bf = mybir.dt.bfloat16
    Act = mybir.ActivationFunctionType
    Alu = mybir.AluOpType

    nc = bacc.Bacc("TRN2", target_bir_lowering=False, debug=False,
                   num_devices=NCORES)

    def din(name, shape, dt):
        return nc.dram_tensor(name, shape, dt, kind="ExternalInput")

    # ---- external inputs (per core) ----
    xT = din("xT", [D, T], bf)           # x^T, feature-major, full
    encT = din("encT", [D, T], bf)       # encoder_output^T, full
    xsh = din("xsh", [TSH, D], f32)      # my token shard of x, + bp1 folded
    wq1 = din("wq1", [D, HPC * DH], bf)  # my heads, pre-scaled by DH^-0.5
    wk1 = din("wk1", [D, HPC * DH], bf)
    wv1 = din("wv1", [D, HPC * DH], bf)
    bq1 = din("bq1", [DH, HPC], f32)     # pre-scaled
    bk1 = din("bk1", [DH, HPC], f32)
    bv1r = din("bv1r", [1, HPC * DH], f32)
    wp1 = din("wp1", [D, D], bf)
    wq2 = din("wq2", [D, HPC * DH], bf)  # pre-scaled
    bq2 = din("bq2", [DH, HPC], f32)
    wk2a = din("wk2a", [D, D], bf)       # all heads
    bk2a = din("bk2a", [DH, H], f32)
    wv2a = din("wv2a", [D, D], bf)
    bv2r = din("bv2r", [1, D], f32)
    wp2 = din("wp2", [D, D], bf)
    bp2r = din("bp2r", [1, D], f32)
    wf1 = din("wf1", [D, DFF], bf)
    bf1c = din("bf1c", [DH, FT], f32)
    wf2 = din("wf2", [DFF, D], bf)
    bf2r = din("bf2r", [1, D], f32)
    g1r = din("g1r", [1, D], f32)
    b1r = din("b1r", [1, D], f32)
    g2r = din("g2r", [1, D], f32)
    b2r = din("b2r", [1, D], f32)
    g3r = din("g3r", [1, D], f32)
    b3r = din("b3r", [1, D], f32)

    osh = nc.dram_tensor("osh", [TSH, D], f32, kind="ExternalOutput")

    taps = {}
    if debug_taps:
        taps["qT1"] = nc.dram_tensor("tap_qT1", [DH, HPC, T], f32, kind="ExternalOutput")
        taps["den1"] = nc.dram_tensor("tap_den1", [DH, 4, 8], f32, kind="ExternalOutput")
        taps["outT1"] = nc.dram_tensor("tap_outT1", [DH, HPC, T], f32, kind="ExternalOutput")
        taps["x1"] = nc.dram_tensor("tap_x1", [TSH, D], f32, kind="ExternalOutput")
        taps["x2"] = nc.dram_tensor("tap_x2", [TSH, D], f32, kind="ExternalOutput")
        taps["kT2"] = nc.dram_tensor("tap_kT2", [DH, HPC, T], f32, kind="ExternalOutput")
        taps["v2"] = nc.dram_tensor("tap_v2", [DH, 16, HPC * DH], f32, kind="ExternalOutput")

    # rearranged DRAM views: [128, tile, free]
    xT3 = xT.ap().rearrange("(o p) t -> p o t", p=128)
    encT3 = encT.ap().rearrange("(o p) t -> p o t", p=128)
    wq1r = wq1.ap().rearrange("(o p) e -> p o e", p=128)
    wk1r = wk1.ap().rearrange("(o p) e -> p o e", p=128)
    wv1r = wv1.ap().rearrange("(o p) e -> p o e", p=128)
    wq2r = wq2.ap().rearrange("(o p) e -> p o e", p=128)
    wk2r = wk2a.ap().rearrange("(o p) f -> p o f", p=128)
    wv2r = wv2a.ap().rearrange("(o p) f -> p o f", p=128)
    wp1r = wp1.ap().rearrange("(o p) d -> p o d", p=128)
    wp2r = wp2.ap().rearrange("(o p) d -> p o d", p=128)
    wf1r = wf1.ap().rearrange("(o p) f -> p o f", p=128)
    wf2r = wf2.ap().rearrange("(o p) d -> p o d", p=128)
    xshr = xsh.ap().rearrange("(o p) d -> p o d", p=128)
    oshr = osh.ap().rearrange("(o p) d -> p o d", p=128)

    def bcast_row(row_t, lo, n, p=128):
        ap = row_t.ap()
        st = ap.ap[-1][0]
        return bass.AP(tensor=ap.tensor, offset=ap.offset + lo * st,
                       ap=[[0, p], [st, n]])

    with tile.TileContext(nc) as tc:
        import contextlib
        ctx = contextlib.ExitStack()
        with ctx:
            pool = ctx.enter_context(tc.tile_pool(name="persist", bufs=1))
            qtp = ctx.enter_context(tc.tile_pool(name="qtp", bufs=1))
            ktp = ctx.enter_context(tc.tile_pool(name="ktp", bufs=1))
            v1p = ctx.enter_context(tc.tile_pool(name="v1p", bufs=1))
            xtp = ctx.enter_context(tc.tile_pool(name="xtp", bufs=1))
            yp = ctx.enter_context(tc.tile_pool(name="yp", bufs=2))
            expp = ctx.enter_context(tc.tile_pool(name="expp", bufs=3))
            htp = ctx.enter_context(tc.tile_pool(name="htp", bufs=2))
            xblk = ctx.enter_context(tc.tile_pool(name="xblk", bufs=2))
            xblk1 = ctx.enter_context(tc.tile_pool(name="xblk1", bufs=3))
            wrow = ctx.enter_context(tc.tile_pool(name="wrow", bufs=2))
            ott = ctx.enter_context(tc.tile_pool(name="ott", bufs=4))
            vsc = ctx.enter_context(tc.tile_pool(name="vsc", bufs=8))
            kv2 = ctx.enter_context(tc.tile_pool(name="kv2", bufs=3))
            bc = ctx.enter_context(tc.tile_pool(name="bc", bufs=2))
            sm = ctx.enter_context(tc.tile_pool(name="sm", bufs=6))
            resid = ctx.enter_context(tc.tile_pool(name="resid", bufs=2))
            ffap = ctx.enter_context(tc.tile_pool(name="ffap", bufs=1))
            ps = ctx.enter_context(tc.tile_pool(name="ps", bufs=7, space="PSUM"))
            psacc = ctx.enter_context(tc.tile_pool(name="psacc", bufs=1, space="PSUM"))
            dram = ctx.enter_context(tc.tile_pool(name="dram", bufs=1, space="DRAM"))

            # ---- constants / persistent weights in SBUF ----
            ident = pool.tile([128, 128], f32)
            make_identity(nc, ident[:])
            eps_sb = pool.tile([128, 1], f32)
            nc.vector.memset(eps_sb[:], 1e-5)

            wv1_sb = pool.tile([128, DT, HPC * DH], bf)
            nc.sync.dma_start(wv1_sb[:], wv1r)
            bq1_sb = pool.tile([128, HPC], f32)
            nc.sync.dma_start(bq1_sb[:], bq1.ap()[:])
            bk1_sb = pool.tile([128, HPC], f32)
            nc.sync.dma_start(bk1_sb[:], bk1.ap()[:])
            bq2_sb = pool.tile([128, HPC], f32)
            nc.sync.dma_start(bq2_sb[:], bq2.ap()[:])
            bk2_sb = pool.tile([128, H], f32)
            nc.sync.dma_start(bk2_sb[:], bk2a.ap()[:])
            bf1_sb = pool.tile([128, FT], f32)
            nc.sync.dma_start(bf1_sb[:], bf1c.ap()[:])
            bv1_sb = pool.tile([128, HPC * DH], f32)
            nc.gpsimd.dma_start(bv1_sb[:], bcast_row(bv1r, 0, HPC * DH))

            # ---- DRAM bounce buffers for the 4 AllToAlls ----
            o1_in = dram.tile([NCORES, HPC * DH, TSH], bf)
            o1_out = dram.tile([NCORES, HPC * DH, TSH], bf)
            kv2_in = dram.tile([NCORES, 2, HPC * DH, TSH], bf)
            kv2_out = dram.tile([NCORES, 2, HPC * DH, TSH], bf)
            o2_in = dram.tile([NCORES, HPC * DH, TSH], bf)
            o2_out = dram.tile([NCORES, HPC * DH, TSH], bf)
            rg = [list(range(NCORES))]

            def a2a(src, dst):
                nc.gpsimd.collective_compute(
                    "AllToAll", Alu.bypass, replica_groups=rg,
                    ins=[src.opt()], outs=[dst.opt()])

            # ================= helpers =================
            def qk_proj(src3, wdram, b_sb, dst, nproj=2, w2dram=None, b2_sb=None,
                        dst2=None):
                """Feature-major projections for my heads over all tokens.
                dst[:, h, t] = sum_d w[d, h*DH+e] * src[d, t] + b[e, h]."""
                for tc4 in range(T // 512):
                    xtb = xblk.tile([128, DT, 512], bf, tag="xblk")
                    nc.sync.dma_start(xtb[:], src3[:, :, tc4 * 512:(tc4 + 1) * 512])
                    wt1 = xblk1.tile([128, DT, HPC * DH], bf, tag="xblk1")
                    nc.gpsimd.dma_start(wt1[:], wdram)
                    plist = [(wt1, b_sb, dst)]
                    if nproj == 2:
                        wt2 = xblk1.tile([128, DT, HPC * DH], bf, tag="xblk1")
                        nc.gpsimd.dma_start(wt2[:], w2dram)
                        plist.append((wt2, b2_sb, dst2))
                    for h in range(HPC):
                        for (wsb, bsb, dd) in plist:
                            pst = ps.tile([128, 512], f32, tag="ps")
                            for dt in range(DT):
                                nc.tensor.matmul(
                                    pst[:], wsb[:, dt, h * DH:(h + 1) * DH],
                                    xtb[:, dt, :],
                                    start=(dt == 0), stop=(dt == DT - 1))
                            nc.vector.tensor_scalar(
                                out=dd[:, h, tc4 * 512:(tc4 + 1) * 512],
                                in0=pst[:], scalar1=bsb[:, h:h + 1],
                                scalar2=None, op0=Alu.add)

            def attn_units(qT, kTs, getv, o_in):
                """Per (b, h): scores^T=[k,q] -> exp (ACT) -> denom (DVE
                reduce) -> scale V rows -> AV -> outT[:, h, b*S+q].
                Units are software-pipelined: unit u+1's scores are issued
                before unit u's AV so the PE stays busy during exp."""
                KT = S // 128   # 8 k tiles per batch
                QC = S // 512   # 2 q chunks per batch
                units = [(b, h) for b in range(B) for h in range(HPC)]

                def scores_phase(b, h):
                    expt = [expp.tile([128, KT, 512], bf, tag="expt",
                                      name=f"exp_{b}_{h}_{qc}")
                            for qc in range(QC)]
                    dred = sm.tile([128, QC, KT], f32, tag="dred")
                    for qc in range(QC):
                        for kc in range(KT):
                            pst = ps.tile([128, 512], f32, tag="ps")
                            nc.tensor.matmul(
                                pst[:],
                                kTs(h, b * KT + kc),
                                qT[:, h, b * S + qc * 512: b * S + (qc + 1) * 512],
                                start=True, stop=True)
                            nc.scalar.activation(
                                out=expt[qc][:, kc, :],
                                in_=pst[:], func=Act.Exp)
                        nc.vector.tensor_reduce(
                            out=dred[:, qc, :], in_=expt[qc][:],
                            axis=mybir.AxisListType.X, op=Alu.add)
                    return expt, dred

                def av_phase(b, h, expt, dred):
                    den = sm.tile([128, KT], f32, tag="den")
                    nc.vector.tensor_tensor(den[:], dred[:, 0, :],
                                            dred[:, 1, :], Alu.add)
                    rden = sm.tile([128, KT], f32, tag="rden")
                    nc.vector.reciprocal(rden[:], den[:])
                    vts = []
                    for kc in range(KT):
                        vt = vsc.tile([128, DH], bf, tag="vsc")
                        nc.vector.tensor_scalar(
                            out=vt[:], in0=getv(h, b * KT + kc),
                            scalar1=rden[:, kc:kc + 1], scalar2=None,
                            op0=Alu.mult)
                        vts.append(vt)
                    for qc in range(QC):
                        pav = psacc.tile([128, 512], f32, tag="psacc")
                        for kc in range(KT):
                            nc.tensor.matmul(
                                pav[:], vts[kc][:],
                                expt[qc][:, kc, :],
                                start=(kc == 0), stop=(kc == KT - 1))
                        ot = kv2.tile([128, 512], bf, tag="kv2")
                        nc.scalar.activation(out=ot[:], in_=pav[:],
                                             func=Act.Copy)
                        j0 = (b * S + qc * 512) // TSH
                        for jj in range(2):
                            nc.sync.dma_start(
                                o_in[j0 + jj, h * DH:(h + 1) * DH, :],
                                ot[:, jj * 256:(jj + 1) * 256])

                pending = None
                for (b, h) in units:
                    cur = (b, h, *scores_phase(b, h))
                    if pending is not None:
                        pb, ph, pe_, pd = pending
                        av_phase(pb, ph, pe_, pd)
                    pending = cur
                pb, ph, pe_, pd = pending
                av_phase(pb, ph, pe_, pd)

            def outproj_ln(o_out, wpr, resid_kind, grow, brow, y_dst, xT_dst,
                           tap=None):
                """Token-sharded out-projection + residual + LN.
                resid_kind: ("xsh",) | ("y", tile, bp_row)."""
                pss = {}
                for et in range(DT):
                    wrb = wrow.tile([128, 2048], bf, tag="wrow")
                    nc.gpsimd.dma_start(wrb[:], wpr[:, et, :])
                    for tc2 in range(TSH // 128):
                        otl = ott.tile([128, 128], bf, tag="ott")
                        nc.gpsimd.dma_start(
                            otl[:],
                            o_out[et // HPC,
                                  (et % HPC) * DH:(et % HPC + 1) * DH,
                                  tc2 * 128:(tc2 + 1) * 128])
                        for dc in range(4):
                            key = (tc2, dc)
                            if et == 0:
                                pl = psacc if key == (1, 3) else ps
                                pss[key] = pl.tile([128, 512], f32,
                                                   tag=pl is psacc and "psacc" or "ps",
                                                   name=f"op_{key}")
                            nc.tensor.matmul(
                                pss[key][:], otl[:],
                                wrb[:, dc * 512:(dc + 1) * 512],
                                start=(et == 0), stop=(et == DT - 1))
                for tc2 in range(TSH // 128):
                    pre = resid.tile([128, D], f32, tag="resid")
                    if resid_kind[0] == "xsh":
                        rsh = resid.tile([128, D], f32, tag="resid")
                        nc.sync.dma_start(rsh[:], xshr[:, tc2, :])
                        for dc in range(4):
                            nc.vector.tensor_tensor(
                                pre[:, dc * 512:(dc + 1) * 512],
                                pss[(tc2, dc)][:],
                                rsh[:, dc * 512:(dc + 1) * 512], Alu.add)
                    else:
                        ybase, bprow = resid_kind[1], resid_kind[2]
                        for dc in range(4):
                            nc.vector.tensor_tensor(
                                pre[:, dc * 512:(dc + 1) * 512],
                                pss[(tc2, dc)][:],
                                ybase[:, tc2, dc * 512:(dc + 1) * 512], Alu.add)
                        for dc in range(4):
                            bpt = bc.tile([128, 512], f32, tag="bc")
                            nc.gpsimd.dma_start(
                                bpt[:], bcast_row(bprow, dc * 512, 512))
                            nc.vector.tensor_tensor(
                                pre[:, dc * 512:(dc + 1) * 512],
                                pre[:, dc * 512:(dc + 1) * 512], bpt[:], Alu.add)
                    ln_apply(pre, grow, brow, tc2, y_dst, xT_dst, None, tap)

            def ln_apply(pre, grow, brow, tc2, y_dst, xT_dst, f32_out, tap):
                stats = sm.tile([128, 4, 6], f32, tag="stats")
                for sg in range(4):
                    nc.vector.bn_stats(stats[:, sg, :],
                                       pre[:, sg * 512:(sg + 1) * 512])
                mv = sm.tile([128, 2], f32, tag="mv")
                nc.vector.bn_aggr(mv[:], stats[:])
                sd = sm.tile([128, 1], f32, tag="sd")
                nc.scalar.activation(sd[:], mv[:, 1:2], Act.Sqrt, bias=eps_sb[:])
                rstd = sm.tile([128, 1], f32, tag="rstd")
                nc.vector.reciprocal(rstd[:], sd[:])
                nmr = sm.tile([128, 1], f32, tag="nmr")
                nc.vector.tensor_tensor(nmr[:], mv[:, 0:1], rstd[:], Alu.mult)
                nc.vector.tensor_scalar_mul(nmr[:], nmr[:], -1.0)
                yf = pre
                nc.scalar.activation(yf[:], pre[:], Act.Identity,
                                     bias=nmr[:], scale=rstd[:])
                for dc in range(4):
                    sl = slice(dc * 512, (dc + 1) * 512)
                    gt = bc.tile([128, 512], f32, tag="bc")
                    nc.gpsimd.dma_start(gt[:], bcast_row(grow, dc * 512, 512))
                    nc.vector.tensor_tensor(yf[:, sl], yf[:, sl], gt[:], Alu.mult)
                    bt = bc.tile([128, 512], f32, tag="bc")
                    nc.gpsimd.dma_start(bt[:], bcast_row(brow, dc * 512, 512))
                    nc.vector.tensor_tensor(yf[:, sl], yf[:, sl], bt[:], Alu.add)
                if tap is not None:
                    nc.sync.dma_start(tap[:, tc2, :], yf[:])
                if y_dst is None:
                    # final LN: write fp32 shard out
                    nc.sync.dma_start(oshr[:, tc2, :], yf[:])
                    return
                nc.vector.tensor_copy(out=y_dst[:, tc2, :], in_=yf[:])
                # transposes -> xT_dst [128, DT, TSH] bf16
                for dt in range(DT):
                    pst = ps.tile([128, 512], f32, tag="ps")
                    nc.tensor.transpose(pst[:, :128],
                                        yf[:, dt * 128:(dt + 1) * 128], ident[:])
                    nc.vector.tensor_copy(
                        out=xT_dst[:, dt, tc2 * 128:(tc2 + 1) * 128],
                        in_=pst[:, :128])

            # ================= layer 1: self-attention =================
            qT1 = qtp.tile([128, HPC, T], bf, tag="qt")
            kT1 = ktp.tile([128, HPC, T], bf, tag="kt")
            qk_proj(xT3, wq1r, bq1_sb, qT1, nproj=2, w2dram=wk1r,
                    b2_sb=bk1_sb, dst2=kT1)

            v1 = v1p.tile([128, 16, HPC * DH], bf, tag="v1")
            for tt in range(T // 128):
                xtb1 = xblk1.tile([128, DT, 128], bf, tag="xblk1")
                nc.sync.dma_start(xtb1[:], xT3[:, :, tt * 128:(tt + 1) * 128])
                pst = ps.tile([128, 512], f32, tag="ps")
                for dt in range(DT):
                    nc.tensor.matmul(pst[:, :HPC * DH], xtb1[:, dt, :],
                                     wv1_sb[:, dt, :],
                                     start=(dt == 0), stop=(dt == DT - 1))
                nc.vector.tensor_tensor(v1[:, tt, :], pst[:, :HPC * DH],
                                        bv1_sb[:], Alu.add)

            attn_units(
                qT1,
                lambda h, tt: kT1[:, h, tt * 128:(tt + 1) * 128],
                lambda h, tt: v1[:, tt, h * DH:(h + 1) * DH],
                o1_in)
            a2a(o1_in, o1_out)

            # cross-attn queries (only depend on encT) - overlaps the A2A
            qT2 = qtp.tile([128, HPC, T], bf, tag="qt")
            qk_proj(encT3, wq2r, bq2_sb, qT2, nproj=1)

            # out-projection 1 + residual + LN1
            y1 = yp.tile([128, TSH // 128, D], bf, tag="y")
            x1T = xtp.tile([128, DT, TSH], bf, tag="xt")
            outproj_ln(o1_out, wp1r, ("xsh",), g1r, b1r, y1, x1T,
                       tap=(taps["x1"].ap().rearrange("(o p) d -> p o d", p=128)
                            if debug_taps else None))

            # ============ layer 2: cross-attention K/V (token-sharded) ====
            for fc in range(H):
                wkb = xblk1.tile([128, DT, 128], bf, tag="xblk1")
                nc.gpsimd.dma_start(wkb[:], wk2r[:, :, fc * 128:(fc + 1) * 128])
                pst = ps.tile([128, 512], f32, tag="ps")
                for dt in range(DT):
                    nc.tensor.matmul(pst[:, :TSH], wkb[:, dt, :], x1T[:, dt, :],
                                     start=(dt == 0), stop=(dt == DT - 1))
                ktl = kv2.tile([128, TSH], bf, tag="kv2")
                nc.vector.tensor_scalar(out=ktl[:], in0=pst[:, :TSH],
                                        scalar1=bk2_sb[:, fc:fc + 1],
                                        scalar2=None, op0=Alu.add)
                nc.sync.dma_start(
                    kv2_in[fc // HPC, 0,
                           (fc % HPC) * DH:(fc % HPC + 1) * DH, :],
                    ktl[:])

            for f4 in range(4):
                wvb = xblk.tile([128, DT, 512], bf, tag="xblk")
                nc.gpsimd.dma_start(wvb[:], wv2r[:, :, f4 * 512:(f4 + 1) * 512])
                for tc2 in range(TSH // 128):
                    pst = ps.tile([128, 512], f32, tag="ps")
                    for dt in range(DT):
                        nc.tensor.matmul(pst[:],
                                         x1T[:, dt, tc2 * 128:(tc2 + 1) * 128],
                                         wvb[:, dt, :],
                                         start=(dt == 0), stop=(dt == DT - 1))
                    bvt = bc.tile([128, 512], f32, tag="bc")
                    nc.gpsimd.dma_start(bvt[:], bcast_row(bv2r, f4 * 512, 512))
                    vtl = kv2.tile([128, 512], bf, tag="kv2")
                    nc.vector.tensor_tensor(vtl[:], pst[:], bvt[:], Alu.add)
                    for jh in range(2):
                        nc.sync.dma_start(
                            kv2_in[f4 * 2 + jh, 1,
                                   tc2 * 128:(tc2 + 1) * 128, :],
                            vtl[:, jh * 256:(jh + 1) * 256])
            a2a(kv2_in, kv2_out)

            # gather my heads' K^T over all tokens
            kT2 = ktp.tile([128, HPC, T], bf, tag="kt")
            for j in range(NCORES):
                for h in range(HPC):
                    nc.sync.dma_start(
                        kT2[:, h, j * TSH:(j + 1) * TSH],
                        kv2_out[j, 0, h * DH:(h + 1) * DH, :])

            def getv2(h, tt):
                vt = vsc.tile([128, DH], bf, tag="v2l")
                nc.sync.dma_start(
                    vt[:], kv2_out[tt // 2, 1,
                                   (tt % 2) * 128:(tt % 2) * 128 + 128,
                                   h * DH:(h + 1) * DH])
                return vt[:]

            attn_units(
                qT2,
                lambda h, tt: kT2[:, h, tt * 128:(tt + 1) * 128],
                getv2,
                o2_in)
            a2a(o2_in, o2_out)

            y2 = yp.tile([128, TSH // 128, D], bf, tag="y")
            x2T = xtp.tile([128, DT, TSH], bf, tag="xt")
            outproj_ln(o2_out, wp2r, ("y", y1, bp2r), g2r, b2r, y2, x2T,
                       tap=(taps["x2"].ap().rearrange("(o p) d -> p o d", p=128)
                            if debug_taps else None))

            # ================= FFN (token-local) =================
            ffacc = ffap.tile([128, TSH // 128, D], bf, tag="ffacc")
            for qtr in range(4):
                hTq = htp.tile([128, 16, TSH], bf, tag="ht")
                for fci in range(16):
                    fc = qtr * 16 + fci
                    wfb = xblk1.tile([128, DT, 128], bf, tag="xblk1")
                    nc.gpsimd.dma_start(wfb[:], wf1r[:, :, fc * 128:(fc + 1) * 128])
                    pst = ps.tile([128, 512], f32, tag="ps")
                    for dt in range(DT):
                        nc.tensor.matmul(pst[:, :TSH], wfb[:, dt, :],
                                         x2T[:, dt, :],
                                         start=(dt == 0), stop=(dt == DT - 1))
                    nc.scalar.activation(out=hTq[:, fci, :], in_=pst[:, :TSH],
                                         func=Act.Relu, bias=bf1_sb[:, fc:fc + 1])
                pss = {}
                for fti in range(16):
                    ft = qtr * 16 + fti
                    wrb = wrow.tile([128, 2048], bf, tag="wrow")
                    nc.gpsimd.dma_start(wrb[:], wf2r[:, ft, :])
                    for tc2 in range(TSH // 128):
                        for dc in range(4):
                            key = (tc2, dc)
                            if fti == 0:
                                pl = psacc if key == (1, 3) else ps
                                pss[key] = pl.tile([128, 512], f32,
                                                   tag=pl is psacc and "psacc" or "ps",
                                                   name=f"ff_{qtr}_{key}")
                            nc.tensor.matmul(
                                pss[key][:],
                                hTq[:, fti, tc2 * 128:(tc2 + 1) * 128],
                                wrb[:, dc * 512:(dc + 1) * 512],
                                start=(fti == 0), stop=(fti == 15))
                for tc2 in range(TSH // 128):
                    for dc in range(4):
                        dsl = slice(dc * 512, (dc + 1) * 512)
                        if qtr == 0:
                            nc.vector.tensor_copy(out=ffacc[:, tc2, dsl],
                                                  in_=pss[(tc2, dc)][:])
                        else:
                            nc.vector.tensor_tensor(
                                ffacc[:, tc2, dsl], ffacc[:, tc2, dsl],
                                pss[(tc2, dc)][:], Alu.add)

            for tc2 in range(TSH // 128):
                pre = resid.tile([128, D], f32, tag="resid")
                for dc in range(4):
                    sl = slice(dc * 512, (dc + 1) * 512)
                    nc.vector.tensor_tensor(pre[:, sl], ffacc[:, tc2, sl],
                                            y2[:, tc2, sl], Alu.add)
                    bft = bc.tile([128, 512], f32, tag="bc")
                    nc.gpsimd.dma_start(bft[:], bcast_row(bf2r, dc * 512, 512))
                    nc.vector.tensor_tensor(pre[:, sl], pre[:, sl], bft[:],
                                            Alu.add)
                ln_apply(pre, g3r, b3r, tc2, None, None, None, None)

            if debug_taps:
                tmp = pool.tile([128, HPC, T], f32)
                nc.vector.tensor_copy(out=tmp[:], in_=qT1[:])
                nc.sync.dma_start(taps["qT1"].ap()[:], tmp[:])

    nc.compile()
    return nc


def _prep_inputs(inputs):
    """Host-side shard/transpose/cast. Returns list of per-core in_maps."""
    g = {k: np.asarray(v, np.float32) for k, v in inputs.items()}
    s = 1.0 / np.sqrt(np.float32(DH))
    xf = g["x"].reshape(T, D)
    encf = g["encoder_output"].reshape(T, D)
    xT = np.ascontiguousarray(xf.T).astype(BF16)
    encT = np.ascontiguousarray(encf.T).astype(BF16)
    shared = {
        "xT": xT, "encT": encT,
        "wk2a": np.ascontiguousarray(
            g["wk2"].transpose(1, 0, 2).reshape(D, D)).astype(BF16),
        "bk2a": np.ascontiguousarray(g["bk2"].T),
        "wv2a": np.ascontiguousarray(
            g["wv2"].transpose(1, 0, 2).reshape(D, D)).astype(BF16),
        "bv2r": g["bv2"].reshape(1, D).copy(),
        "wp1": g["wp1"].astype(BF16),
        "wp2": g["wp2"].astype(BF16),
        "bp2r": g["bp2"].reshape(1, D).copy(),
        "wf1": g["w_ff1"].astype(BF16),
        "bf1c": np.ascontiguousarray(g["b_ff1"].reshape(FT, DH).T),
        "wf2": g["w_ff2"].astype(BF16),
        "bf2r": g["b_ff2"].reshape(1, D).copy(),
        "g1r": g["ln1_g"].reshape(1, D).copy(),
        "b1r": g["ln1_b"].reshape(1, D).copy(),
        "g2r": g["ln2_g"].reshape(1, D).copy(),
        "b2r": g["ln2_b"].reshape(1, D).copy(),
        "g3r": g["ln3_g"].reshape(1, D).copy(),
        "b3r": g["ln3_b"].reshape(1, D).copy(),
    }
    in_maps = []
    for c in range(NCORES):
        hs = slice(HPC * c, HPC * (c + 1))
        m = dict(shared)
        m["xsh"] = xf[TSH * c: TSH * (c + 1)] + g["bp1"][None, :]
        m["wq1"] = np.ascontiguousarray(
            g["wq1"][hs].transpose(1, 0, 2).reshape(D, HPC * DH) * s).astype(BF16)
        m["wk1"] = np.ascontiguousarray(
            g["wk1"][hs].transpose(1, 0, 2).reshape(D, HPC * DH)).astype(BF16)
        m["wv1"] = np.ascontiguousarray(
            g["wv1"][hs].transpose(1, 0, 2).reshape(D, HPC * DH)).astype(BF16)
        m["bq1"] = np.ascontiguousarray(g["bq1"][hs].T * s)
        m["bk1"] = np.ascontiguousarray(g["bk1"][hs].T)
        m["bv1r"] = g["bv1"][hs].reshape(1, HPC * DH).copy()
        m["wq2"] = np.ascontiguousarray(
            g["wq2"][hs].transpose(1, 0, 2).reshape(D, HPC * DH) * s).astype(BF16)
        m["bq2"] = np.ascontiguousarray(g["bq2"][hs].T * s)
        in_maps.append(m)
    return in_maps


def kernel(**inputs):
    from concourse import bass_utils
    if "prog" not in _CACHE:
        _CACHE["prog"] = _build()
    nc = _CACHE["prog"]
    in_maps = _prep_inputs(inputs)
    res = bass_utils.run_bass_kernel_spmd(
        nc, in_maps, core_ids=list(range(NCORES)))
    _CACHE["last_result"] = res
    out = np.concatenate([res.results[c]["osh"] for c in range(NCORES)], axis=0)
    return out.reshape(B, S, D).astype(np.float32)
